# revision 9
# baseline (speedup 1.0000x reference)
# Bass/Trainium2 kernel for MHConvAttention (B=16, C=128, H=W=64, NH=8, OUT=512)
# Data-parallel over batch: 8 cores x 2 samples each.
#
# Per-sample layout: channels (128) on SBUF partitions, flattened spatial (4096)
# on the free dim. Depthwise convs run as fp8e4 DoubleRow tap-pair matmuls
# (diagonal weights, 2 taps per instruction); the CPE center tap + residual
# stays exact via one f32r matmul per chunk. The content-lambda path uses a
# transposed QKV GEMM (spatial-on-partitions); the ECA channel-attention is
# folded into the out-projection weights; the out projection is fused into the
# per-chunk loop so output DMA drains throughout instead of bunching at the
# tail.
import os
import numpy as np

B, C, H, W = 16, 128, 64, 64
NH, HD, WIN, OUT = 8, 16, 5, 512
N = H * W
NCORES = 8
SPC = B // NCORES          # samples per core
NC8 = N // 512             # 512-wide chunks per sample
NJ = N // 128              # 128-wide chunks (transposed GEMM)
SCALING = HD ** (-0.5)

_CACHE = {}

# 5x5 depthwise conv: 12 fp8-DoubleRow tap pairs + single tap 24 (paired with
# zero weights). Pair members must sit at a constant element offset in the
# padded v buffer (pitch W+4).
_P5 = ([((dy, 0), (dy, 1)) for dy in range(5)]
       + [((dy, 2), (dy, 3)) for dy in range(5)]
       + [((0, 4), (1, 4)), ((2, 4), (3, 4))])
_P5_SINGLE = (4, 4)

# CPE 3x3: 4 fp8-DoubleRow pairs covering the 8 non-center taps (pitch W+2);
# the center tap plus the +src residual run as one exact f32r matmul.
_P3 = [((0, 0), (0, 1)), ((0, 2), (1, 0)), ((1, 2), (2, 0)), ((2, 1), (2, 2))]


def _build_nc():
    import concourse.bass as bass
    import concourse.tile as tile
    import concourse.mybir as mybir
    from concourse import bacc

    f32 = mybir.dt.float32
    f32r = mybir.dt.float32r
    bf16 = mybir.dt.bfloat16
    fp8 = mybir.dt.float8e4
    DR = mybir.MatmulPerfMode.DoubleRow
    Alu = mybir.AluOpType
    Act = mybir.ActivationFunctionType

    def r(ap):
        return ap.bitcast(f32r)

    def pair_ap(sl, delta):
        # insert a k-tile dim of size 2 with the given element stride after
        # the partition dim: [P, a, b] -> [P, 2, a, b]
        dims = [list(sl.ap[0]), [delta, 2]] + [list(d) for d in sl.ap[1:]]
        return bass.AP(sl.tensor, sl.offset, dims)

    nc = bacc.Bacc(trn_type="TRN2", target_bir_lowering=False, debug=False)

    src_d = nc.dram_tensor("src", [SPC, C, H, W], f32, kind="ExternalInput").ap()
    d3_d = nc.dram_tensor("d3", [C, 8, C], fp8, kind="ExternalInput").ap()
    dctr_d = nc.dram_tensor("dctr", [C, C], f32, kind="ExternalInput").ap()
    d5_d = nc.dram_tensor("d5", [C, 26, C], fp8, kind="ExternalInput").ap()
    wq_d = nc.dram_tensor("wq", [C, C], f32, kind="ExternalInput").ap()
    wv_d = nc.dram_tensor("wv", [C, C], f32, kind="ExternalInput").ap()
    wkv_d = nc.dram_tensor("wkv", [C, 2 * C], f32, kind="ExternalInput").ap()
    w1_d = nc.dram_tensor("w1", [C, OUT], f32, kind="ExternalInput").ap()
    w2_d = nc.dram_tensor("w2", [C, OUT], f32, kind="ExternalInput").ap()
    mask_d = nc.dram_tensor("mask", [C, C], f32, kind="ExternalInput").ap()
    trid_d = nc.dram_tensor("trid", [C, C], f32, kind="ExternalInput").ap()
    out_d = nc.dram_tensor("out", [SPC, OUT, H, W], f32, kind="ExternalOutput").ap()
    out_v = out_d.rearrange("s o h w -> s o (h w)")

    with tile.TileContext(nc) as tc, __import__("contextlib").ExitStack() as ctx:
        wpool = ctx.enter_context(tc.tile_pool(name="w", bufs=1))
        sraw_pool = ctx.enter_context(tc.tile_pool(name="sraw", bufs=2))
        sp8_pool = ctx.enter_context(tc.tile_pool(name="sp8", bufs=2))
        s_pool = ctx.enter_context(tc.tile_pool(name="s", bufs=8))
        q_pool = ctx.enter_context(tc.tile_pool(name="q", bufs=8))
        r1_pool = ctx.enter_context(tc.tile_pool(name="r1", bufs=2))
        vpad_pool = ctx.enter_context(tc.tile_pool(name="vpad", bufs=1))
        eT_pool = ctx.enter_context(tc.tile_pool(name="eT", bufs=1))
        vT_pool = ctx.enter_context(tc.tile_pool(name="vT", bufs=1))
        tmp_pool = ctx.enter_context(tc.tile_pool(name="tmp", bufs=2))
        stage_pool = ctx.enter_context(tc.tile_pool(name="stage", bufs=6))
        w2p_pool = ctx.enter_context(tc.tile_pool(name="w2p", bufs=1))
        small_pool = ctx.enter_context(tc.tile_pool(name="small", bufs=2))
        ps_pool = ctx.enter_context(tc.tile_pool(name="ps", bufs=4, space="PSUM"))
        psT_pool = ctx.enter_context(tc.tile_pool(name="psT", bufs=2, space="PSUM"))
        psCL_pool = ctx.enter_context(tc.tile_pool(name="psCL", bufs=1, space="PSUM"))
        psE_pool = ctx.enter_context(tc.tile_pool(name="psE", bufs=1, space="PSUM"))

        # ---- loads in order of first use; src as big contiguous DMAs ----
        d3_sb = wpool.tile([C, 8, C], fp8)
        nc.sync.dma_start(d3_sb[:], d3_d[:])
        dctr_sb = wpool.tile([C, C], f32)
        nc.sync.dma_start(r(dctr_sb[:]), r(dctr_d[:]))
        sraw_t = []
        for smp in range(SPC):
            sr = sraw_pool.tile([C, H, W], f32, tag="sraw")
            nc.sync.dma_start(r(sr[:]), r(src_d[smp]))
            sraw_t.append(sr)
        wq_sb = wpool.tile([C, C], f32)
        nc.sync.dma_start(r(wq_sb[:]), r(wq_d[:]))
        wv_sb = wpool.tile([C, C], f32)
        nc.sync.dma_start(r(wv_sb[:]), r(wv_d[:]))
        wkv_sb = wpool.tile([C, 2 * C], f32)
        nc.sync.dma_start(r(wkv_sb[:]), r(wkv_d[:]))
        d5_sb = wpool.tile([C, 26, C], fp8)
        nc.sync.dma_start(d5_sb[:], d5_d[:])
        w1_sb = wpool.tile([C, OUT], f32)
        nc.sync.dma_start(r(w1_sb[:]), r(w1_d[:]))
        w2_sb = wpool.tile([C, OUT], f32)
        nc.sync.dma_start(w2_sb[:], w2_d[:])
        mask_sb = wpool.tile([C, C], f32)
        nc.sync.dma_start(mask_sb[:], mask_d[:])
        trid_sb = wpool.tile([C, C], f32)
        nc.sync.dma_start(trid_sb[:], trid_d[:])

        for smp in range(SPC):
            sraw = sraw_t[smp]

            # fp8 padded copy of src for the CPE DoubleRow taps
            sp8 = sp8_pool.tile([C, H + 2, W + 2], fp8, tag="sp8")
            nc.vector.memset(sp8[:, 0:1, :], 0.0)
            nc.vector.memset(sp8[:, H + 1 : H + 2, :], 0.0)
            nc.vector.memset(sp8[:, :, 0:1], 0.0)
            nc.vector.memset(sp8[:, :, W + 1 : W + 2], 0.0)
            nc.scalar.copy(sp8[:, 1 : H + 1, 1 : W + 1], sraw[:])

            # ---- CPE 3x3: 4 fp8 DR pairs + exact f32r center(+residual) ----
            s_t = []
            for c8 in range(NC8):
                ps = ps_pool.tile([C, 512], f32, tag="ps")
                y0 = 8 * c8
                for p, ((dy0, dx0), (dy1, dx1)) in enumerate(_P3):
                    sl = sp8[:, y0 + dy0 : y0 + dy0 + 8, dx0 : dx0 + W]
                    delta = (dy1 - dy0) * (W + 2) + (dx1 - dx0)
                    nc.tensor.matmul(
                        ps[:], d3_sb[:, 2 * p : 2 * p + 2, :],
                        pair_ap(sl, delta), start=(p == 0), stop=False,
                        perf_mode=DR,
                    )
                nc.tensor.matmul(
                    ps[:], r(dctr_sb[:]), r(sraw[:, y0 : y0 + 8, :]),
                    start=False, stop=True,
                )
                st = s_pool.tile([C, 512], f32, tag="s")
                if c8 % 2 == 0:
                    nc.vector.tensor_copy(r(st[:]), ps[:])
                else:
                    nc.scalar.copy(r(st[:]), ps[:])
                s_t.append(st)

            # ---- forward QKV GEMM: q and v in channels-on-partitions layout ----
            vpad = vpad_pool.tile([C, H + 4, W + 4], fp8, tag="vpad")
            nc.vector.memset(vpad[:, 0:2, :], 0.0)
            nc.vector.memset(vpad[:, H + 2 : H + 4, :], 0.0)
            nc.vector.memset(vpad[:, :, 0:2], 0.0)
            nc.vector.memset(vpad[:, :, W + 2 : W + 4], 0.0)
            q_t = []
            for c8 in range(NC8):
                psq = ps_pool.tile([C, 512], f32, tag="ps")
                nc.tensor.matmul(psq[:], r(wq_sb[:]), r(s_t[c8][:]), start=True, stop=True)
                qt = q_pool.tile([C, 512], f32, tag="q")
                if c8 % 2 == 0:
                    nc.scalar.copy(r(qt[:]), psq[:])
                else:
                    nc.vector.tensor_copy(r(qt[:]), psq[:])
                q_t.append(qt)
                psv = ps_pool.tile([C, 512], f32, tag="ps")
                nc.tensor.matmul(psv[:], r(wv_sb[:]), r(s_t[c8][:]), start=True, stop=True)
                nc.vector.tensor_copy(
                    vpad[:, 2 + 8 * c8 : 2 + 8 * c8 + 8, 2 : W + 2],
                    psv[:].rearrange("p (a b) -> p a b", a=8),
                )

            # ---- transposed GEMM: [kT | vT] chunks; exp(kT) -> eT, vT + ones col ----
            eT = eT_pool.tile([C, NJ, C], bf16, tag="eT")
            vT = vT_pool.tile([C, NJ, C + 1], bf16, tag="vT")
            nc.vector.memset(vT[:, :, C : C + 1], 1.0)
            for j in range(NJ):
                psT = psT_pool.tile([C, 2 * C], f32, tag="psT")
                lhs = s_t[j // 4][:, (j % 4) * 128 : (j % 4 + 1) * 128]
                nc.tensor.matmul(psT[:], r(lhs), r(wkv_sb[:]), start=True, stop=True)
                nc.scalar.activation(eT[:, j, :], psT[:, 0:C], Act.Exp)
                nc.vector.tensor_copy(vT[:, j, 0:C], psT[:, C : 2 * C])

            # ---- ECA: ca = sigmoid(tridiag @ mean_pool(src)) ----
            pool_sum = small_pool.tile([C, 1], f32, tag="psum_vec")
            nc.vector.reduce_sum(pool_sum[:], sraw[:], axis=mybir.AxisListType.XY)
            ps_eca = psE_pool.tile([C, 1], f32, tag="pse")
            nc.tensor.matmul(ps_eca[:], trid_sb[:], pool_sum[:], start=True, stop=True)
            ca = small_pool.tile([C, 1], f32, tag="ca")
            nc.scalar.activation(ca[:], ps_eca[:], Act.Sigmoid)
            w2p = w2p_pool.tile([C, OUT], f32, tag="w2p")
            nc.vector.tensor_scalar(r(w2p[:]), w2_sb[:], ca[:], None, Alu.mult)

            # ---- content lambda: CL[i, o] (+ row sums in col 128) ----
            ps_cl = psCL_pool.tile([C, C + 1], f32, tag="pscl")
            for j in range(NJ):
                nc.tensor.matmul(
                    ps_cl[:], eT[:, j, :], vT[:, j, :],
                    start=(j == 0), stop=(j == NJ - 1),
                )
            recip = small_pool.tile([C, 1], f32, tag="recip")
            nc.vector.reciprocal(recip[:], ps_cl[:, C : C + 1])
            cln_t = small_pool.tile([C, C], f32, tag="cln_t")
            nc.vector.tensor_scalar(cln_t[:], ps_cl[:, 0:C], recip[:], None, Alu.mult)
            cln = small_pool.tile([C, C], f32, tag="cln")
            nc.vector.tensor_tensor(r(cln[:]), cln_t[:], mask_sb[:], Alu.mult)

            # ---- per chunk: 5x5 conv + content output + r1 + out projection ----
            for c8 in range(NC8):
                ps5 = ps_pool.tile([C, 512], f32, tag="ps")
                y0 = 8 * c8
                for p, ((dy0, dx0), (dy1, dx1)) in enumerate(_P5):
                    sl = vpad[:, y0 + dy0 : y0 + dy0 + 8, dx0 : dx0 + W]
                    delta = (dy1 - dy0) * (W + 4) + (dx1 - dx0)
                    nc.tensor.matmul(
                        ps5[:], d5_sb[:, 2 * p : 2 * p + 2, :],
                        pair_ap(sl, delta), start=(p == 0), stop=False,
                        perf_mode=DR,
                    )
                dy, dx = _P5_SINGLE
                sl = vpad[:, y0 + dy : y0 + dy + 8, dx : dx + W]
                nc.tensor.matmul(
                    ps5[:], d5_sb[:, 24:26, :], pair_ap(sl, -1),
                    start=False, stop=True, perf_mode=DR,
                )
                psc = ps_pool.tile([C, 512], f32, tag="ps")
                nc.tensor.matmul(psc[:], r(cln[:]), r(q_t[c8][:]), start=True, stop=True)
                tmp = tmp_pool.tile([C, 512], f32, tag="tmp")
                nc.vector.tensor_tensor(tmp[:], q_t[c8][:], ps5[:], Alu.mult)
                rt = r1_pool.tile([C, 512], f32, tag="r1")
                nc.vector.tensor_tensor(r(rt[:]), tmp[:], psc[:], Alu.add)

                for m in range(OUT // C):
                    pso = ps_pool.tile([C, 512], f32, tag="ps")
                    nc.tensor.matmul(
                        pso[:], r(w1_sb[:, m * C : (m + 1) * C]), r(rt[:]),
                        start=True, stop=False,
                    )
                    nc.tensor.matmul(
                        pso[:], r(w2p[:, m * C : (m + 1) * C]),
                        r(sraw[:, y0 : y0 + 8, :]),
                        start=False, stop=True,
                    )
                    stg = stage_pool.tile([C, 512], f32, tag="stage")
                    if m % 2 == 0:
                        nc.scalar.copy(stg[:], pso[:])
                    else:
                        nc.vector.tensor_copy(stg[:], pso[:])
                    eng = nc.sync if m % 2 == 0 else nc.scalar
                    eng.dma_start(
                        out_v[smp, m * C : (m + 1) * C, c8 * 512 : (c8 + 1) * 512],
                        stg[:],
                    )

    nc.compile()
    return nc


def _get_nc():
    if "nc" not in _CACHE:
        _CACHE["nc"] = _build_nc()
    return _CACHE["nc"]


def _host_weights(cpe_w, qkv_w, rel_pos, conv1d_w, out_w):
    cpe_w = np.asarray(cpe_w, np.float32)
    qkv_w = np.asarray(qkv_w, np.float32)
    rel_pos = np.asarray(rel_pos, np.float32)
    conv1d_w = np.asarray(conv1d_w, np.float32)
    out_w = np.asarray(out_w, np.float32)

    import ml_dtypes
    idx = np.arange(C)

    d3 = np.zeros([C, 8, C], ml_dtypes.float8_e4m3)
    taps3 = [t for pr in _P3 for t in pr]
    for slot, (dy, dx) in enumerate(taps3):
        d3[idx, slot, idx] = cpe_w[:, 0, dy, dx]
    dctr = np.zeros([C, C], np.float32)
    dctr[idx, idx] = 1.0 + cpe_w[:, 0, 1, 1]  # residual + center tap, exact

    d5 = np.zeros([C, 26, C], ml_dtypes.float8_e4m3)
    taps5 = [t for pr in _P5 for t in pr] + [_P5_SINGLE]
    for slot, (dy, dx) in enumerate(taps5):
        d5[idx, slot, idx] = rel_pos[idx % HD, dy, dx]

    wq = np.ascontiguousarray(qkv_w[0:C, :].T)
    wv = np.ascontiguousarray(qkv_w[2 * C : 3 * C, :].T)
    wkv = np.ascontiguousarray(qkv_w[C : 3 * C, :].T)
    w1 = np.ascontiguousarray(out_w[:, 0:C].T)
    w2 = np.ascontiguousarray(out_w[:, C : 2 * C].T)

    mask = np.zeros([C, C], np.float32)
    for h in range(NH):
        mask[h * HD : (h + 1) * HD, h * HD : (h + 1) * HD] = SCALING

    trid = np.zeros([C, C], np.float32)
    trid[idx[:-1], idx[:-1] + 1] = conv1d_w[0]  # pool[c-1] contributes to ca[c]
    trid[idx, idx] = conv1d_w[1]
    trid[idx[1:], idx[1:] - 1] = conv1d_w[2]
    trid *= 1.0 / N
    return dict(d3=d3, dctr=dctr, d5=d5, wq=wq, wv=wv, wkv=wkv, w1=w1, w2=w2,
                mask=mask, trid=trid)


def kernel(src, cpe_w, qkv_w, rel_pos, conv1d_w, out_w):
    from concourse.bass_utils import run_bass_kernel_spmd

    src = np.asarray(src, np.float32)
    w = _host_weights(cpe_w, qkv_w, rel_pos, conv1d_w, out_w)
    nc = _get_nc()
    in_maps = [
        {"src": np.ascontiguousarray(src[i * SPC : (i + 1) * SPC]), **w}
        for i in range(NCORES)
    ]
    trace = bool(os.environ.get("BASS_TRACE"))
    res = run_bass_kernel_spmd(nc, in_maps, list(range(NCORES)), trace=trace)
    _CACHE["last_result"] = res
    out = np.concatenate([res.results[i]["out"] for i in range(NCORES)], axis=0)
    return out


# revision 17
# speedup vs baseline: 1.1513x; 1.1513x over previous
# Bass/Trainium2 kernel for MHConvAttention (B=16, C=128, H=W=64, NH=8, OUT=512)
# Data-parallel over batch: 8 cores x 2 samples each.
#
# Per-sample layout: channels (128) on SBUF partitions, flattened spatial (4096)
# on the free dim. Depthwise convs run as fp8e4 DoubleRow tap-pair matmuls
# (diagonal weights, 2 taps per instruction); the CPE center tap + residual
# stays exact via one f32r matmul per chunk. The content-lambda path uses a
# transposed QKV GEMM (spatial-on-partitions); the ECA channel-attention is
# folded into the out-projection weights; the out projection is fused into the
# per-chunk loop so output DMA drains throughout instead of bunching at the
# tail.
import os
import numpy as np

B, C, H, W = 16, 128, 64, 64
NH, HD, WIN, OUT = 8, 16, 5, 512
N = H * W
NCORES = 8
SPC = B // NCORES          # samples per core
NC8 = N // 512             # 512-wide chunks per sample
NJ = N // 128              # 128-wide chunks (transposed GEMM)
SCALING = HD ** (-0.5)

_CACHE = {}

# 5x5 depthwise conv: 12 fp8-DoubleRow tap pairs + single tap 24 (paired with
# zero weights). Pair members must sit at a constant element offset in the
# padded v buffer (pitch W+4).
_P5 = ([((dy, 0), (dy, 1)) for dy in range(5)]
       + [((dy, 2), (dy, 3)) for dy in range(5)]
       + [((0, 4), (1, 4)), ((2, 4), (3, 4))])
_P5_SINGLE = (4, 4)

# CPE 3x3: 4 fp8-DoubleRow pairs covering the 8 non-center taps (pitch W+2);
# the center tap plus the +src residual run as one exact f32r matmul.
_P3 = [((0, 0), (0, 1)), ((0, 2), (1, 0)), ((1, 2), (2, 0)), ((2, 1), (2, 2))]


def _build_nc():
    import concourse.bass as bass
    import concourse.tile as tile
    import concourse.mybir as mybir
    from concourse import bacc

    f32 = mybir.dt.float32
    f32r = mybir.dt.float32r
    bf16 = mybir.dt.bfloat16
    fp8 = mybir.dt.float8e4
    DR = mybir.MatmulPerfMode.DoubleRow
    Alu = mybir.AluOpType
    Act = mybir.ActivationFunctionType

    def r(ap):
        return ap.bitcast(f32r)

    def pair_ap(sl, delta):
        # insert a k-tile dim of size 2 with the given element stride after
        # the partition dim: [P, a, b] -> [P, 2, a, b]
        dims = [list(sl.ap[0]), [delta, 2]] + [list(d) for d in sl.ap[1:]]
        return bass.AP(sl.tensor, sl.offset, dims)

    nc = bacc.Bacc(trn_type="TRN2", target_bir_lowering=False, debug=False)

    src_d = nc.dram_tensor("src", [SPC, C, H, W], f32, kind="ExternalInput").ap()
    src8_d = nc.dram_tensor("src8", [SPC, C, H + 2, W + 2], fp8,
                            kind="ExternalInput").ap()
    d3_d = nc.dram_tensor("d3", [C, 8, C], fp8, kind="ExternalInput").ap()
    dctr_d = nc.dram_tensor("dctr", [C, C], f32, kind="ExternalInput").ap()
    d5_d = nc.dram_tensor("d5", [C, 26, C], fp8, kind="ExternalInput").ap()
    wq_d = nc.dram_tensor("wq", [C, C], f32, kind="ExternalInput").ap()
    wv_d = nc.dram_tensor("wv", [C, C], f32, kind="ExternalInput").ap()
    wkv_d = nc.dram_tensor("wkv", [C, 2 * C], f32, kind="ExternalInput").ap()
    w1_d = nc.dram_tensor("w1", [C, OUT], f32, kind="ExternalInput").ap()
    w2_d = nc.dram_tensor("w2", [C, OUT], f32, kind="ExternalInput").ap()
    mask_d = nc.dram_tensor("mask", [C, C], f32, kind="ExternalInput").ap()
    trid_d = nc.dram_tensor("trid", [C, C], f32, kind="ExternalInput").ap()
    out_d = nc.dram_tensor("out", [SPC, OUT, H, W], f32, kind="ExternalOutput").ap()
    out_v = out_d.rearrange("s o h w -> s o (h w)")

    with tile.TileContext(nc) as tc, __import__("contextlib").ExitStack() as ctx:
        wpool = ctx.enter_context(tc.tile_pool(name="w", bufs=1))
        sraw_pool = ctx.enter_context(tc.tile_pool(name="sraw", bufs=2))
        sp8_pool = ctx.enter_context(tc.tile_pool(name="sp8", bufs=2))
        s_pool = ctx.enter_context(tc.tile_pool(name="s", bufs=8))
        q_pool = ctx.enter_context(tc.tile_pool(name="q", bufs=8))
        r1_pool = ctx.enter_context(tc.tile_pool(name="r1", bufs=2))
        vpad_pool = ctx.enter_context(tc.tile_pool(name="vpad", bufs=1))
        eT_pool = ctx.enter_context(tc.tile_pool(name="eT", bufs=1))
        vT_pool = ctx.enter_context(tc.tile_pool(name="vT", bufs=1))
        tmp_pool = ctx.enter_context(tc.tile_pool(name="tmp", bufs=2))
        stage_pool = ctx.enter_context(tc.tile_pool(name="stage", bufs=6))
        w2p_pool = ctx.enter_context(tc.tile_pool(name="w2p", bufs=1))
        small_pool = ctx.enter_context(tc.tile_pool(name="small", bufs=2))
        ps_pool = ctx.enter_context(tc.tile_pool(name="ps", bufs=5, space="PSUM"))
        psT_pool = ctx.enter_context(tc.tile_pool(name="psT", bufs=2, space="PSUM"))
        psCL_pool = ctx.enter_context(tc.tile_pool(name="psCL", bufs=1, space="PSUM"))

        # ---- loads in order of first use; src as big contiguous DMAs ----
        d3_sb = wpool.tile([C, 8, C], fp8)
        nc.sync.dma_start(d3_sb[:], d3_d[:])
        dctr_sb = wpool.tile([C, C], f32)
        nc.sync.dma_start(r(dctr_sb[:]), r(dctr_d[:]))
        sp8_t = []
        sraw_t = []
        for smp in range(SPC):
            sp8_tile = sp8_pool.tile([C, H + 2, W + 2], fp8, tag="sp8")
            sp8_t.append(sp8_tile)
            sraw_tile = sraw_pool.tile([C, H, W], f32, tag="sraw")
            sraw_t.append(sraw_tile)
        nc.sync.dma_start(sp8_t[0][:], src8_d[0])
        hh = H // 2
        nc.sync.dma_start(r(sraw_t[0][:, 0:hh]), r(src_d[0, :, 0:hh]))
        nc.sync.dma_start(r(sraw_t[0][:, hh:H]), r(src_d[0, :, hh:H]))
        wq_sb = wpool.tile([C, C], f32)
        nc.sync.dma_start(r(wq_sb[:]), r(wq_d[:]))
        wv_sb = wpool.tile([C, C], f32)
        nc.sync.dma_start(r(wv_sb[:]), r(wv_d[:]))
        wkv_sb = wpool.tile([C, 2 * C], f32)
        nc.sync.dma_start(r(wkv_sb[:]), r(wkv_d[:]))
        d5_sb = wpool.tile([C, 26, C], fp8)
        nc.sync.dma_start(d5_sb[:], d5_d[:])
        w1_sb = wpool.tile([C, OUT], f32)
        nc.sync.dma_start(r(w1_sb[:]), r(w1_d[:]))
        w2_sb = wpool.tile([C, OUT], f32)
        nc.sync.dma_start(w2_sb[:], w2_d[:])
        mask_sb = wpool.tile([C, C], f32)
        nc.sync.dma_start(mask_sb[:], mask_d[:])
        trid_sb = wpool.tile([C, C], f32)
        nc.sync.dma_start(trid_sb[:], trid_d[:])
        # second sample's src arrives behind the weights
        nc.sync.dma_start(sp8_t[1][:], src8_d[1])
        nc.sync.dma_start(r(sraw_t[1][:, 0:hh]), r(src_d[1, :, 0:hh]))
        nc.sync.dma_start(r(sraw_t[1][:, hh:H]), r(src_d[1, :, hh:H]))

        for smp in range(SPC):
            sraw = sraw_t[smp]
            sp8 = sp8_t[smp]

            # ---- CPE 3x3: 4 fp8 DR pairs + exact f32r center(+residual) ----
            s_t = []
            for c8 in range(NC8):
                ps = ps_pool.tile([C, 512], f32, tag="ps")
                y0 = 8 * c8
                for p, ((dy0, dx0), (dy1, dx1)) in enumerate(_P3):
                    sl = sp8[:, y0 + dy0 : y0 + dy0 + 8, dx0 : dx0 + W]
                    delta = (dy1 - dy0) * (W + 2) + (dx1 - dx0)
                    nc.tensor.matmul(
                        ps[:], d3_sb[:, 2 * p : 2 * p + 2, :],
                        pair_ap(sl, delta), start=(p == 0), stop=False,
                        perf_mode=DR,
                    )
                nc.tensor.matmul(
                    ps[:], r(dctr_sb[:]), r(sraw[:, y0 : y0 + 8, :]),
                    start=False, stop=True,
                )
                st = s_pool.tile([C, 512], f32, tag="s")
                if c8 % 2 == 0:
                    nc.vector.tensor_copy(r(st[:]), ps[:])
                else:
                    nc.scalar.copy(r(st[:]), ps[:])
                s_t.append(st)

            # ---- forward QKV GEMM: q and v in channels-on-partitions layout ----
            vpad = vpad_pool.tile([C, H + 4, W + 4], fp8, tag="vpad")
            nc.vector.memset(vpad[:, 0:2, :], 0.0)
            nc.vector.memset(vpad[:, H + 2 : H + 4, :], 0.0)
            nc.vector.memset(vpad[:, :, 0:2], 0.0)
            nc.vector.memset(vpad[:, :, W + 2 : W + 4], 0.0)
            q_t = []
            for c8 in range(NC8):
                psq = ps_pool.tile([C, 512], f32, tag="ps")
                nc.tensor.matmul(psq[:], r(wq_sb[:]), r(s_t[c8][:]), start=True, stop=True)
                qt = q_pool.tile([C, 512], f32, tag="q")
                if c8 % 2 == 0:
                    nc.scalar.copy(r(qt[:]), psq[:])
                else:
                    nc.vector.tensor_copy(r(qt[:]), psq[:])
                q_t.append(qt)
                psv = ps_pool.tile([C, 512], f32, tag="ps")
                nc.tensor.matmul(psv[:], r(wv_sb[:]), r(s_t[c8][:]), start=True, stop=True)
                nc.vector.tensor_copy(
                    vpad[:, 2 + 8 * c8 : 2 + 8 * c8 + 8, 2 : W + 2],
                    psv[:].rearrange("p (a b) -> p a b", a=8),
                )

            # ---- transposed GEMM: [kT | vT] chunks; exp(kT) -> eT, vT + ones col ----
            eT = eT_pool.tile([C, NJ, C], bf16, tag="eT")
            vT = vT_pool.tile([C, NJ, C + 1], bf16, tag="vT")
            nc.vector.memset(vT[:, :, C : C + 1], 1.0)
            for j in range(NJ):
                psT = psT_pool.tile([C, 2 * C], f32, tag="psT")
                lhs = s_t[j // 4][:, (j % 4) * 128 : (j % 4 + 1) * 128]
                nc.tensor.matmul(psT[:], r(lhs), r(wkv_sb[:]), start=True, stop=True)
                nc.scalar.activation(eT[:, j, :], psT[:, 0:C], Act.Exp)
                nc.vector.tensor_copy(vT[:, j, 0:C], psT[:, C : 2 * C])

            # ---- ECA: ca = sigmoid(tridiag @ mean_pool(src)) ----
            pool_sum = small_pool.tile([C, 1], f32, tag="psum_vec")
            nc.vector.reduce_sum(pool_sum[:], sraw[:], axis=mybir.AxisListType.XY)
            ps_eca = ps_pool.tile([C, 512], f32, tag="ps")
            nc.tensor.matmul(ps_eca[:, 0:1], trid_sb[:], pool_sum[:], start=True, stop=True)
            ca = small_pool.tile([C, 1], f32, tag="ca")
            nc.scalar.activation(ca[:], ps_eca[:, 0:1], Act.Sigmoid)
            w2p = w2p_pool.tile([C, OUT], f32, tag="w2p")
            nc.vector.tensor_scalar(r(w2p[:]), w2_sb[:], ca[:], None, Alu.mult)

            # ---- content lambda: CL[i, o] (+ row sums in col 128) ----
            ps_cl = psCL_pool.tile([C, C + 1], f32, tag="pscl")
            for j in range(NJ):
                nc.tensor.matmul(
                    ps_cl[:], eT[:, j, :], vT[:, j, :],
                    start=(j == 0), stop=(j == NJ - 1),
                )
            recip = small_pool.tile([C, 1], f32, tag="recip")
            nc.vector.reciprocal(recip[:], ps_cl[:, C : C + 1])
            cln_t = small_pool.tile([C, C], f32, tag="cln_t")
            nc.vector.tensor_scalar(cln_t[:], ps_cl[:, 0:C], recip[:], None, Alu.mult)
            cln = small_pool.tile([C, C], f32, tag="cln")
            nc.vector.tensor_tensor(r(cln[:]), cln_t[:], mask_sb[:], Alu.mult)

            # ---- per chunk: 5x5 conv + content output + r1 + out projection,
            # software-pipelined so out-proj(c8-1) overlaps the DVE r1 chain ----
            def emit_outproj(rt, c8):
                y0 = 8 * c8
                for m in range(OUT // C):
                    pso = ps_pool.tile([C, 512], f32, tag="ps")
                    nc.tensor.matmul(
                        pso[:], r(w1_sb[:, m * C : (m + 1) * C]), r(rt[:]),
                        start=True, stop=False,
                    )
                    nc.tensor.matmul(
                        pso[:], r(w2p[:, m * C : (m + 1) * C]),
                        r(sraw[:, y0 : y0 + 8, :]),
                        start=False, stop=True,
                    )
                    stg = stage_pool.tile([C, 512], f32, tag="stage")
                    if m % 2 == 0:
                        nc.scalar.copy(stg[:], pso[:])
                    else:
                        nc.vector.tensor_copy(stg[:], pso[:])
                    eng = nc.sync if m % 2 == 0 else nc.scalar
                    eng.dma_start(
                        out_v[smp, m * C : (m + 1) * C, c8 * 512 : (c8 + 1) * 512],
                        stg[:],
                    )

            prev = None
            for c8 in range(NC8):
                ps5 = ps_pool.tile([C, 512], f32, tag="ps")
                y0 = 8 * c8
                for p, ((dy0, dx0), (dy1, dx1)) in enumerate(_P5):
                    sl = vpad[:, y0 + dy0 : y0 + dy0 + 8, dx0 : dx0 + W]
                    delta = (dy1 - dy0) * (W + 4) + (dx1 - dx0)
                    nc.tensor.matmul(
                        ps5[:], d5_sb[:, 2 * p : 2 * p + 2, :],
                        pair_ap(sl, delta), start=(p == 0), stop=False,
                        perf_mode=DR,
                    )
                dy, dx = _P5_SINGLE
                sl = vpad[:, y0 + dy : y0 + dy + 8, dx : dx + W]
                nc.tensor.matmul(
                    ps5[:], d5_sb[:, 24:26, :], pair_ap(sl, -1),
                    start=False, stop=True, perf_mode=DR,
                )
                psc = ps_pool.tile([C, 512], f32, tag="ps")
                nc.tensor.matmul(psc[:], r(cln[:]), r(q_t[c8][:]), start=True, stop=True)
                tmp = tmp_pool.tile([C, 512], f32, tag="tmp")
                nc.vector.tensor_tensor(tmp[:], q_t[c8][:], ps5[:], Alu.mult)
                rt = r1_pool.tile([C, 512], f32, tag="r1")
                nc.vector.tensor_tensor(r(rt[:]), tmp[:], psc[:], Alu.add)
                if prev is not None:
                    emit_outproj(*prev)
                prev = (rt, c8)
            emit_outproj(*prev)

    nc.compile()
    return nc


def _get_nc():
    if "nc" not in _CACHE:
        _CACHE["nc"] = _build_nc()
    return _CACHE["nc"]


def _host_weights(cpe_w, qkv_w, rel_pos, conv1d_w, out_w):
    cpe_w = np.asarray(cpe_w, np.float32)
    qkv_w = np.asarray(qkv_w, np.float32)
    rel_pos = np.asarray(rel_pos, np.float32)
    conv1d_w = np.asarray(conv1d_w, np.float32)
    out_w = np.asarray(out_w, np.float32)

    import ml_dtypes
    idx = np.arange(C)

    d3 = np.zeros([C, 8, C], ml_dtypes.float8_e4m3)
    taps3 = [t for pr in _P3 for t in pr]
    for slot, (dy, dx) in enumerate(taps3):
        d3[idx, slot, idx] = cpe_w[:, 0, dy, dx]
    dctr = np.zeros([C, C], np.float32)
    dctr[idx, idx] = 1.0 + cpe_w[:, 0, 1, 1]  # residual + center tap, exact

    d5 = np.zeros([C, 26, C], ml_dtypes.float8_e4m3)
    taps5 = [t for pr in _P5 for t in pr] + [_P5_SINGLE]
    for slot, (dy, dx) in enumerate(taps5):
        d5[idx, slot, idx] = rel_pos[idx % HD, dy, dx]

    wq = np.ascontiguousarray(qkv_w[0:C, :].T)
    wv = np.ascontiguousarray(qkv_w[2 * C : 3 * C, :].T)
    wkv = np.ascontiguousarray(qkv_w[C : 3 * C, :].T)
    w1 = np.ascontiguousarray(out_w[:, 0:C].T)
    w2 = np.ascontiguousarray(out_w[:, C : 2 * C].T)

    mask = np.zeros([C, C], np.float32)
    for h in range(NH):
        mask[h * HD : (h + 1) * HD, h * HD : (h + 1) * HD] = SCALING

    trid = np.zeros([C, C], np.float32)
    trid[idx[:-1], idx[:-1] + 1] = conv1d_w[0]  # pool[c-1] contributes to ca[c]
    trid[idx, idx] = conv1d_w[1]
    trid[idx[1:], idx[1:] - 1] = conv1d_w[2]
    trid *= 1.0 / N
    return dict(d3=d3, dctr=dctr, d5=d5, wq=wq, wv=wv, wkv=wkv, w1=w1, w2=w2,
                mask=mask, trid=trid)


def kernel(src, cpe_w, qkv_w, rel_pos, conv1d_w, out_w):
    from concourse.bass_utils import run_bass_kernel_spmd

    import ml_dtypes

    src = np.asarray(src, np.float32)
    w = _host_weights(cpe_w, qkv_w, rel_pos, conv1d_w, out_w)
    nc = _get_nc()
    src8 = np.zeros([B, C, H + 2, W + 2], ml_dtypes.float8_e4m3)
    src8[:, :, 1 : H + 1, 1 : W + 1] = src
    in_maps = [
        {"src": np.ascontiguousarray(src[i * SPC : (i + 1) * SPC]),
         "src8": np.ascontiguousarray(src8[i * SPC : (i + 1) * SPC]), **w}
        for i in range(NCORES)
    ]
    trace = bool(os.environ.get("BASS_TRACE"))
    res = run_bass_kernel_spmd(nc, in_maps, list(range(NCORES)), trace=trace)
    _CACHE["last_result"] = res
    out = np.concatenate([res.results[i]["out"] for i in range(NCORES)], axis=0)
    return out


# revision 31
# speedup vs baseline: 1.1605x; 1.0081x over previous
# Bass/Trainium2 kernel for MHConvAttention (B=16, C=128, H=W=64, NH=8, OUT=512)
# Data-parallel over batch: 8 cores x 2 samples each.
#
# Per-sample layout: channels (128) on SBUF partitions, flattened spatial (4096)
# on the free dim. Depthwise convs run as fp8e4 DoubleRow tap-pair matmuls
# (diagonal weights, 2 taps per instruction); the CPE center tap + residual
# stays exact via one f32r matmul per chunk. The content-lambda path uses a
# transposed QKV GEMM (spatial-on-partitions); the ECA channel-attention is
# folded into the out-projection weights; the out projection is fused into the
# per-chunk loop so output DMA drains throughout instead of bunching at the
# tail.
import os
import numpy as np

B, C, H, W = 16, 128, 64, 64
NH, HD, WIN, OUT = 8, 16, 5, 512
N = H * W
NCORES = 8
SPC = B // NCORES          # samples per core
NC8 = N // 512             # 512-wide chunks per sample
NJ = N // 128              # 128-wide chunks (transposed GEMM)
SCALING = HD ** (-0.5)

_CACHE = {}

# 5x5 depthwise conv: 12 fp8-DoubleRow tap pairs + single tap 24 (paired with
# zero weights). Pair members must sit at a constant element offset in the
# padded v buffer (pitch W+4).
_P5 = ([((dy, 0), (dy, 1)) for dy in range(5)]
       + [((dy, 2), (dy, 3)) for dy in range(5)]
       + [((0, 4), (1, 4)), ((2, 4), (3, 4))])
_P5_SINGLE = (4, 4)

# CPE 3x3: 4 fp8-DoubleRow pairs covering the 8 non-center taps (pitch W+2);
# the center tap plus the +src residual run as one exact f32r matmul.
_P3 = [((0, 0), (0, 1)), ((0, 2), (1, 0)), ((1, 2), (2, 0)), ((2, 1), (2, 2))]


def _build_nc():
    import concourse.bass as bass
    import concourse.tile as tile
    import concourse.mybir as mybir
    from concourse import bacc

    f32 = mybir.dt.float32
    f32r = mybir.dt.float32r
    bf16 = mybir.dt.bfloat16
    fp8 = mybir.dt.float8e4
    DR = mybir.MatmulPerfMode.DoubleRow
    Alu = mybir.AluOpType
    Act = mybir.ActivationFunctionType

    def r(ap):
        return ap.bitcast(f32r)

    def pair_ap(sl, delta):
        # insert a k-tile dim of size 2 with the given element stride after
        # the partition dim: [P, a, b] -> [P, 2, a, b]
        dims = [list(sl.ap[0]), [delta, 2]] + [list(d) for d in sl.ap[1:]]
        return bass.AP(sl.tensor, sl.offset, dims)

    nc = bacc.Bacc(trn_type="TRN2", target_bir_lowering=False, debug=False)

    src_d = nc.dram_tensor("src", [SPC, C, H, W], f32, kind="ExternalInput").ap()
    src8_d = nc.dram_tensor("src8", [SPC, C, H + 2, W + 2], fp8,
                            kind="ExternalInput").ap()
    # weights consolidated into 3 tensors (one dma_start each):
    # wfp8: d3 pair slots 0..7 | d5 pair slots 8..33
    wfp8_d = nc.dram_tensor("wfp8", [C, 34, C], fp8, kind="ExternalInput").ap()
    # wf32a: dctr | wq | wv | wkv ; wf32b: w1 | w2 | mask | trid
    wf32a_d = nc.dram_tensor("wf32a", [C, 5 * C], f32, kind="ExternalInput").ap()
    wf32b_d = nc.dram_tensor("wf32b", [C, 10 * C], f32, kind="ExternalInput").ap()
    out_d = nc.dram_tensor("out", [SPC, OUT, H, W], f32, kind="ExternalOutput").ap()
    out_v = out_d.rearrange("s o h w -> s o (h w)")
    # chunk-consolidated out view: [sample, partition-channel, m-block, n]
    out_m = out_d.rearrange("s (m c) h w -> s c m (h w)", m=OUT // C)

    with tile.TileContext(nc) as tc, __import__("contextlib").ExitStack() as ctx:
        wpool = ctx.enter_context(tc.tile_pool(name="w", bufs=1))
        sraw_pool = ctx.enter_context(tc.tile_pool(name="sraw", bufs=2))
        sp8_pool = ctx.enter_context(tc.tile_pool(name="sp8", bufs=2))
        s_pool = ctx.enter_context(tc.tile_pool(name="s", bufs=8))
        q_pool = ctx.enter_context(tc.tile_pool(name="q", bufs=8))
        r1_pool = ctx.enter_context(tc.tile_pool(name="r1", bufs=2))
        vpad_pool = ctx.enter_context(tc.tile_pool(name="vpad", bufs=1))
        eT_pool = ctx.enter_context(tc.tile_pool(name="eT", bufs=1))
        vT_pool = ctx.enter_context(tc.tile_pool(name="vT", bufs=1))
        tmp_pool = ctx.enter_context(tc.tile_pool(name="tmp", bufs=2))
        stage_pool = ctx.enter_context(tc.tile_pool(name="stage", bufs=6))
        w2p_pool = ctx.enter_context(tc.tile_pool(name="w2p", bufs=1))
        small_pool = ctx.enter_context(tc.tile_pool(name="small", bufs=2))
        ps_pool = ctx.enter_context(tc.tile_pool(name="ps", bufs=5, space="PSUM"))
        psT_pool = ctx.enter_context(tc.tile_pool(name="psT", bufs=2, space="PSUM"))
        psCL_pool = ctx.enter_context(tc.tile_pool(name="psCL", bufs=1, space="PSUM"))

        # ---- loads in order of first use; few dma_starts (issue cost ~1.4us
        # of DIRECT2D descriptor generation each) ----
        wfp8 = wpool.tile([C, 34, C], fp8)
        nc.sync.dma_start(wfp8[:], wfp8_d[:])
        d3_sb = wfp8[:, 0:8]
        d5_sb = wfp8[:, 8:34]
        sp8_t = []
        sraw_t = []
        for smp in range(SPC):
            sp8_tile = sp8_pool.tile([C, H + 2, W + 2], fp8, tag="sp8")
            sp8_t.append(sp8_tile)
            sraw_tile = sraw_pool.tile([C, H, W], f32, tag="sraw")
            sraw_t.append(sraw_tile)
        nc.sync.dma_start(sp8_t[0][:], src8_d[0])
        wf32a = wpool.tile([C, 5 * C], f32)
        nc.sync.dma_start(r(wf32a[:]), r(wf32a_d[:]))
        dctr_sb = wf32a[:, 0:C]
        wq_sb = wf32a[:, C : 2 * C]
        wv_sb = wf32a[:, 2 * C : 3 * C]
        wkv_sb = wf32a[:, 3 * C : 5 * C]
        hh = H // 2
        nc.sync.dma_start(r(sraw_t[0][:, 0:hh]), r(src_d[0, :, 0:hh]))
        nc.sync.dma_start(r(sraw_t[0][:, hh:H]), r(src_d[0, :, hh:H]))
        wf32b = wpool.tile([C, 10 * C], f32)
        nc.sync.dma_start(r(wf32b[:]), r(wf32b_d[:]))
        w1_sb = wf32b[:, 0:OUT]
        w2_sb = wf32b[:, OUT : 2 * OUT]
        mask_sb = wf32b[:, 2 * OUT : 2 * OUT + C]
        trid_sb = wf32b[:, 2 * OUT + C : 2 * OUT + 2 * C]
        # second sample's src arrives behind the weights
        nc.sync.dma_start(sp8_t[1][:], src8_d[1])
        nc.sync.dma_start(r(sraw_t[1][:, 0:hh]), r(src_d[1, :, 0:hh]))
        nc.sync.dma_start(r(sraw_t[1][:, hh:H]), r(src_d[1, :, hh:H]))

        for smp in range(SPC):
            sraw = sraw_t[smp]
            sp8 = sp8_t[smp]

            # ---- CPE 3x3: 4 fp8 DR pairs + exact f32r center(+residual) ----
            s_t = []
            for c8 in range(NC8):
                ps = ps_pool.tile([C, 512], f32, tag="ps")
                y0 = 8 * c8
                for p, ((dy0, dx0), (dy1, dx1)) in enumerate(_P3):
                    sl = sp8[:, y0 + dy0 : y0 + dy0 + 8, dx0 : dx0 + W]
                    delta = (dy1 - dy0) * (W + 2) + (dx1 - dx0)
                    nc.tensor.matmul(
                        ps[:], d3_sb[:, 2 * p : 2 * p + 2, :],
                        pair_ap(sl, delta), start=(p == 0), stop=False,
                        perf_mode=DR,
                    )
                nc.tensor.matmul(
                    ps[:], r(dctr_sb[:]), r(sraw[:, y0 : y0 + 8, :]),
                    start=False, stop=True,
                )
                st = s_pool.tile([C, 512], f32, tag="s")
                if c8 % 2 == 0:
                    nc.vector.tensor_copy(r(st[:]), ps[:])
                else:
                    nc.scalar.copy(r(st[:]), ps[:])
                s_t.append(st)

            # ---- forward QKV GEMM: q and v in channels-on-partitions layout ----
            vpad = vpad_pool.tile([C, H + 4, W + 4], fp8, tag="vpad")
            nc.vector.memset(vpad[:, 0:2, :], 0.0)
            nc.vector.memset(vpad[:, H + 2 : H + 4, :], 0.0)
            nc.vector.memset(vpad[:, :, 0:2], 0.0)
            nc.vector.memset(vpad[:, :, W + 2 : W + 4], 0.0)
            q_t = []
            for c8 in range(NC8):
                psq = ps_pool.tile([C, 512], f32, tag="ps")
                nc.tensor.matmul(psq[:], r(wq_sb[:]), r(s_t[c8][:]), start=True, stop=True)
                qt = q_pool.tile([C, 512], f32, tag="q")
                if c8 % 2 == 0:
                    nc.scalar.copy(r(qt[:]), psq[:])
                else:
                    nc.vector.tensor_copy(r(qt[:]), psq[:])
                q_t.append(qt)
                psv = ps_pool.tile([C, 512], f32, tag="ps")
                nc.tensor.matmul(psv[:], r(wv_sb[:]), r(s_t[c8][:]), start=True, stop=True)
                nc.vector.tensor_copy(
                    vpad[:, 2 + 8 * c8 : 2 + 8 * c8 + 8, 2 : W + 2],
                    psv[:].rearrange("p (a b) -> p a b", a=8),
                )

            # ---- transposed GEMM: [kT | vT] chunks; exp(kT) -> eT, vT + ones col ----
            eT = eT_pool.tile([C, NJ, C], bf16, tag="eT")
            vT = vT_pool.tile([C, NJ, C + 1], bf16, tag="vT")
            nc.vector.memset(vT[:, :, C : C + 1], 1.0)
            for j in range(NJ):
                psT = psT_pool.tile([C, 2 * C], f32, tag="psT")
                lhs = s_t[j // 4][:, (j % 4) * 128 : (j % 4 + 1) * 128]
                nc.tensor.matmul(psT[:], r(lhs), r(wkv_sb[:]), start=True, stop=True)
                nc.scalar.activation(eT[:, j, :], psT[:, 0:C], Act.Exp)
                nc.vector.tensor_copy(vT[:, j, 0:C], psT[:, C : 2 * C])
            # ---- ECA: ca = sigmoid(tridiag @ mean_pool(src)) ----
            pool_sum = small_pool.tile([C, 1], f32, tag="psum_vec")
            nc.vector.reduce_sum(pool_sum[:], sraw[:], axis=mybir.AxisListType.XY)
            ps_eca = ps_pool.tile([C, 512], f32, tag="ps")
            nc.tensor.matmul(ps_eca[:, 0:1], trid_sb[:], pool_sum[:], start=True, stop=True)
            ca = small_pool.tile([C, 1], f32, tag="ca")
            nc.scalar.activation(ca[:], ps_eca[:, 0:1], Act.Sigmoid)
            w2p = w2p_pool.tile([C, OUT], f32, tag="w2p")
            nc.vector.tensor_scalar(r(w2p[:]), w2_sb[:], ca[:], None, Alu.mult)

            # ---- content lambda: CL[i, o] (+ row sums in col 128) ----
            ps_cl = psCL_pool.tile([C, C + 1], f32, tag="pscl")
            for j in range(NJ):
                nc.tensor.matmul(
                    ps_cl[:], eT[:, j, :], vT[:, j, :],
                    start=(j == 0), stop=(j == NJ - 1),
                )
            recip = small_pool.tile([C, 1], f32, tag="recip")
            nc.vector.reciprocal(recip[:], ps_cl[:, C : C + 1])
            cln_t = small_pool.tile([C, C], f32, tag="cln_t")
            nc.vector.tensor_scalar(cln_t[:], ps_cl[:, 0:C], recip[:], None, Alu.mult)
            cln = small_pool.tile([C, C], f32, tag="cln")
            nc.vector.tensor_tensor(r(cln[:]), cln_t[:], mask_sb[:], Alu.mult)

            # ---- per chunk: 5x5 conv + content output + r1 + out projection,
            # software-pipelined so out-proj(c8-1) overlaps the DVE r1 chain ----
            def emit_outproj(rt, c8):
                y0 = 8 * c8
                for m in range(OUT // C):
                    pso = ps_pool.tile([C, 512], f32, tag="ps")
                    nc.tensor.matmul(
                        pso[:], r(wf32b[:, m * C : (m + 1) * C]), r(rt[:]),
                        start=True, stop=False,
                    )
                    nc.tensor.matmul(
                        pso[:], r(w2p[:, m * C : (m + 1) * C]),
                        r(sraw[:, y0 : y0 + 8, :]),
                        start=False, stop=True,
                    )
                    stg = stage_pool.tile([C, 512], f32, tag="stage")
                    if m % 2 == 0:
                        nc.scalar.copy(stg[:], pso[:])
                    else:
                        nc.vector.tensor_copy(stg[:], pso[:])
                    eng = nc.sync if m % 2 == 0 else nc.scalar
                    eng.dma_start(
                        out_v[smp, m * C : (m + 1) * C,
                              c8 * 512 : (c8 + 1) * 512],
                        stg[:],
                    )

            prev = None
            for c8 in range(NC8):
                ps5 = ps_pool.tile([C, 512], f32, tag="ps")
                y0 = 8 * c8
                for p, ((dy0, dx0), (dy1, dx1)) in enumerate(_P5):
                    sl = vpad[:, y0 + dy0 : y0 + dy0 + 8, dx0 : dx0 + W]
                    delta = (dy1 - dy0) * (W + 4) + (dx1 - dx0)
                    nc.tensor.matmul(
                        ps5[:], d5_sb[:, 2 * p : 2 * p + 2, :],
                        pair_ap(sl, delta), start=(p == 0), stop=False,
                        perf_mode=DR,
                    )
                dy, dx = _P5_SINGLE
                sl = vpad[:, y0 + dy : y0 + dy + 8, dx : dx + W]
                nc.tensor.matmul(
                    ps5[:], d5_sb[:, 24:26, :], pair_ap(sl, -1),
                    start=False, stop=True, perf_mode=DR,
                )
                psc = ps_pool.tile([C, 512], f32, tag="ps")
                nc.tensor.matmul(psc[:], r(cln[:]), r(q_t[c8][:]), start=True, stop=True)
                tmp = tmp_pool.tile([C, 512], f32, tag="tmp")
                nc.vector.tensor_tensor(tmp[:], q_t[c8][:], ps5[:], Alu.mult)
                rt = r1_pool.tile([C, 512], f32, tag="r1")
                nc.vector.tensor_tensor(r(rt[:]), tmp[:], psc[:], Alu.add)
                if prev is not None:
                    emit_outproj(*prev)
                prev = (rt, c8)
            emit_outproj(*prev)

    nc.compile()
    return nc


def _get_nc():
    if "nc" not in _CACHE:
        _CACHE["nc"] = _build_nc()
    return _CACHE["nc"]


def _host_weights(cpe_w, qkv_w, rel_pos, conv1d_w, out_w):
    cpe_w = np.asarray(cpe_w, np.float32)
    qkv_w = np.asarray(qkv_w, np.float32)
    rel_pos = np.asarray(rel_pos, np.float32)
    conv1d_w = np.asarray(conv1d_w, np.float32)
    out_w = np.asarray(out_w, np.float32)

    import ml_dtypes
    idx = np.arange(C)

    # wfp8: d3 pair slots 0..7 | d5 pair slots 8..33
    wfp8 = np.zeros([C, 34, C], ml_dtypes.float8_e4m3)
    taps3 = [t for pr in _P3 for t in pr]
    for slot, (dy, dx) in enumerate(taps3):
        wfp8[idx, slot, idx] = cpe_w[:, 0, dy, dx]
    taps5 = [t for pr in _P5 for t in pr] + [_P5_SINGLE]
    for slot, (dy, dx) in enumerate(taps5):
        wfp8[idx, 8 + slot, idx] = rel_pos[idx % HD, dy, dx]

    dctr = np.zeros([C, C], np.float32)
    dctr[idx, idx] = 1.0 + cpe_w[:, 0, 1, 1]  # residual + center tap, exact

    wq = qkv_w[0:C, :].T
    wv = qkv_w[2 * C : 3 * C, :].T
    wkv = qkv_w[C : 3 * C, :].T
    w1 = out_w[:, 0:C].T
    w2 = out_w[:, C : 2 * C].T

    mask = np.zeros([C, C], np.float32)
    for h in range(NH):
        mask[h * HD : (h + 1) * HD, h * HD : (h + 1) * HD] = SCALING

    trid = np.zeros([C, C], np.float32)
    trid[idx[:-1], idx[:-1] + 1] = conv1d_w[0]  # pool[c-1] contributes to ca[c]
    trid[idx, idx] = conv1d_w[1]
    trid[idx[1:], idx[1:] - 1] = conv1d_w[2]
    trid *= 1.0 / N

    wf32a = np.ascontiguousarray(
        np.concatenate([dctr, wq, wv, wkv], axis=1), np.float32)
    wf32b = np.ascontiguousarray(
        np.concatenate([w1, w2, mask, trid], axis=1), np.float32)
    return dict(wfp8=wfp8, wf32a=wf32a, wf32b=wf32b)


def kernel(src, cpe_w, qkv_w, rel_pos, conv1d_w, out_w):
    from concourse.bass_utils import run_bass_kernel_spmd

    import ml_dtypes

    src = np.asarray(src, np.float32)
    w = _host_weights(cpe_w, qkv_w, rel_pos, conv1d_w, out_w)
    nc = _get_nc()
    src8 = np.zeros([B, C, H + 2, W + 2], ml_dtypes.float8_e4m3)
    src8[:, :, 1 : H + 1, 1 : W + 1] = src
    in_maps = [
        {"src": np.ascontiguousarray(src[i * SPC : (i + 1) * SPC]),
         "src8": np.ascontiguousarray(src8[i * SPC : (i + 1) * SPC]), **w}
        for i in range(NCORES)
    ]
    trace = bool(os.environ.get("BASS_TRACE"))
    res = run_bass_kernel_spmd(nc, in_maps, list(range(NCORES)), trace=trace)
    _CACHE["last_result"] = res
    out = np.concatenate([res.results[i]["out"] for i in range(NCORES)], axis=0)
    return out


# revision 34
# speedup vs baseline: 1.1674x; 1.0059x over previous
# Bass/Trainium2 kernel for MHConvAttention (B=16, C=128, H=W=64, NH=8, OUT=512)
# Data-parallel over batch: 8 cores x 2 samples each.
#
# Per-sample layout: channels (128) on SBUF partitions, flattened spatial (4096)
# on the free dim. Depthwise convs run as fp8e4 DoubleRow tap-pair matmuls
# (diagonal weights, 2 taps per instruction); the CPE center tap + residual
# stays exact via one f32r matmul per chunk. The content-lambda path uses a
# transposed QKV GEMM (spatial-on-partitions); the ECA channel-attention is
# folded into the out-projection weights; the out projection is fused into the
# per-chunk loop so output DMA drains throughout instead of bunching at the
# tail.
import os
import numpy as np

B, C, H, W = 16, 128, 64, 64
NH, HD, WIN, OUT = 8, 16, 5, 512
N = H * W
NCORES = 8
SPC = B // NCORES          # samples per core
NC8 = N // 512             # 512-wide chunks per sample
NJ = N // 128              # 128-wide chunks (transposed GEMM)
SCALING = HD ** (-0.5)

_CACHE = {}

# 5x5 depthwise conv: 12 fp8-DoubleRow tap pairs + single tap 24 (paired with
# zero weights). Pair members must sit at a constant element offset in the
# padded v buffer (pitch W+4).
_P5 = ([((dy, 0), (dy, 1)) for dy in range(5)]
       + [((dy, 2), (dy, 3)) for dy in range(5)]
       + [((0, 4), (1, 4)), ((2, 4), (3, 4))])
_P5_SINGLE = (4, 4)

# CPE 3x3: 4 fp8-DoubleRow pairs covering the 8 non-center taps (pitch W+2);
# the center tap plus the +src residual run as one exact f32r matmul.
_P3 = [((0, 0), (0, 1)), ((0, 2), (1, 0)), ((1, 2), (2, 0)), ((2, 1), (2, 2))]


def _build_nc():
    import concourse.bass as bass
    import concourse.tile as tile
    import concourse.mybir as mybir
    from concourse import bacc

    f32 = mybir.dt.float32
    f32r = mybir.dt.float32r
    bf16 = mybir.dt.bfloat16
    fp8 = mybir.dt.float8e4
    DR = mybir.MatmulPerfMode.DoubleRow
    Alu = mybir.AluOpType
    Act = mybir.ActivationFunctionType

    def r(ap):
        return ap.bitcast(f32r)

    def pair_ap(sl, delta):
        # insert a k-tile dim of size 2 with the given element stride after
        # the partition dim: [P, a, b] -> [P, 2, a, b]
        dims = [list(sl.ap[0]), [delta, 2]] + [list(d) for d in sl.ap[1:]]
        return bass.AP(sl.tensor, sl.offset, dims)

    nc = bacc.Bacc(trn_type="TRN2", target_bir_lowering=False, debug=False)

    src_d = nc.dram_tensor("src", [SPC, C, H, W], f32, kind="ExternalInput").ap()
    src8_d = nc.dram_tensor("src8", [SPC, C, H + 2, W + 2], fp8,
                            kind="ExternalInput").ap()
    # weights consolidated into 3 tensors (one dma_start each):
    # wfp8: d3 pair slots 0..7 | d5 pair slots 8..33
    wfp8_d = nc.dram_tensor("wfp8", [C, 34, C], fp8, kind="ExternalInput").ap()
    # wf32a: dctr | wq | wv | wkv ; wf32b: w1 | w2 | mask | trid
    wf32a_d = nc.dram_tensor("wf32a", [C, 5 * C], f32, kind="ExternalInput").ap()
    wf32b_d = nc.dram_tensor("wf32b", [C, 10 * C], f32, kind="ExternalInput").ap()
    out_d = nc.dram_tensor("out", [SPC, OUT, H, W], f32, kind="ExternalOutput").ap()
    out_v = out_d.rearrange("s o h w -> s o (h w)")
    # chunk-consolidated out view: [sample, partition-channel, m-block, n]
    out_m = out_d.rearrange("s (m c) h w -> s c m (h w)", m=OUT // C)

    with tile.TileContext(nc) as tc, __import__("contextlib").ExitStack() as ctx:
        wpool = ctx.enter_context(tc.tile_pool(name="w", bufs=1))
        sraw_pool = ctx.enter_context(tc.tile_pool(name="sraw", bufs=2))
        sp8_pool = ctx.enter_context(tc.tile_pool(name="sp8", bufs=2))
        s_pool = ctx.enter_context(tc.tile_pool(name="s", bufs=8))
        q_pool = ctx.enter_context(tc.tile_pool(name="q", bufs=8))
        r1_pool = ctx.enter_context(tc.tile_pool(name="r1", bufs=2))
        vpad_pool = ctx.enter_context(tc.tile_pool(name="vpad", bufs=1))
        eT_pool = ctx.enter_context(tc.tile_pool(name="eT", bufs=1))
        vT_pool = ctx.enter_context(tc.tile_pool(name="vT", bufs=1))
        tmp_pool = ctx.enter_context(tc.tile_pool(name="tmp", bufs=2))
        stage_pool = ctx.enter_context(tc.tile_pool(name="stage", bufs=6))
        w2p_pool = ctx.enter_context(tc.tile_pool(name="w2p", bufs=1))
        small_pool = ctx.enter_context(tc.tile_pool(name="small", bufs=2))
        ps_pool = ctx.enter_context(tc.tile_pool(name="ps", bufs=5, space="PSUM"))
        psT_pool = ctx.enter_context(tc.tile_pool(name="psT", bufs=2, space="PSUM"))
        psCL_pool = ctx.enter_context(tc.tile_pool(name="psCL", bufs=1, space="PSUM"))

        # ---- loads in order of first use; few dma_starts (issue cost ~1.4us
        # of DIRECT2D descriptor generation each) ----
        wfp8 = wpool.tile([C, 34, C], fp8)
        nc.sync.dma_start(wfp8[:], wfp8_d[:])
        d3_sb = wfp8[:, 0:8]
        d5_sb = wfp8[:, 8:34]
        sp8_t = []
        sraw_t = []
        for smp in range(SPC):
            sp8_tile = sp8_pool.tile([C, H + 2, W + 2], fp8, tag="sp8")
            sp8_t.append(sp8_tile)
            sraw_tile = sraw_pool.tile([C, H, W], f32, tag="sraw")
            sraw_t.append(sraw_tile)
        nc.sync.dma_start(sp8_t[0][:], src8_d[0])
        wf32a = wpool.tile([C, 5 * C], f32)
        nc.sync.dma_start(r(wf32a[:]), r(wf32a_d[:]))
        dctr_sb = wf32a[:, 0:C]
        wq_sb = wf32a[:, C : 2 * C]
        wv_sb = wf32a[:, 2 * C : 3 * C]
        wkv_sb = wf32a[:, 3 * C : 5 * C]
        hh = H // 2
        nc.sync.dma_start(r(sraw_t[0][:, 0:hh]), r(src_d[0, :, 0:hh]))
        nc.sync.dma_start(r(sraw_t[0][:, hh:H]), r(src_d[0, :, hh:H]))
        wf32b = wpool.tile([C, 10 * C], f32)
        nc.sync.dma_start(r(wf32b[:]), r(wf32b_d[:]))
        w1_sb = wf32b[:, 0:OUT]
        w2_sb = wf32b[:, OUT : 2 * OUT]
        mask_sb = wf32b[:, 2 * OUT : 2 * OUT + C]
        trid_sb = wf32b[:, 2 * OUT + C : 2 * OUT + 2 * C]
        # second sample's src arrives behind the weights
        nc.sync.dma_start(sp8_t[1][:], src8_d[1])
        nc.sync.dma_start(r(sraw_t[1][:, 0:hh]), r(src_d[1, :, 0:hh]))
        nc.sync.dma_start(r(sraw_t[1][:, hh:H]), r(src_d[1, :, hh:H]))

        for smp in range(SPC):
            sraw = sraw_t[smp]
            sp8 = sp8_t[smp]

            # ---- CPE 3x3: 4 fp8 DR pairs + exact f32r center(+residual) ----
            s_t = []
            for c8 in range(NC8):
                ps = ps_pool.tile([C, 512], f32, tag="ps")
                y0 = 8 * c8
                for p, ((dy0, dx0), (dy1, dx1)) in enumerate(_P3):
                    sl = sp8[:, y0 + dy0 : y0 + dy0 + 8, dx0 : dx0 + W]
                    delta = (dy1 - dy0) * (W + 2) + (dx1 - dx0)
                    nc.tensor.matmul(
                        ps[:], d3_sb[:, 2 * p : 2 * p + 2, :],
                        pair_ap(sl, delta), start=(p == 0), stop=False,
                        perf_mode=DR,
                    )
                nc.tensor.matmul(
                    ps[:], r(dctr_sb[:]), r(sraw[:, y0 : y0 + 8, :]),
                    start=False, stop=True,
                )
                st = s_pool.tile([C, 512], f32, tag="s")
                if c8 % 2 == 0:
                    nc.vector.tensor_copy(r(st[:]), ps[:])
                else:
                    nc.scalar.copy(r(st[:]), ps[:])
                s_t.append(st)

            # ---- ECA: ca = sigmoid(tridiag @ mean_pool(src)) ----
            pool_sum = small_pool.tile([C, 1], f32, tag="psum_vec")
            nc.vector.reduce_sum(pool_sum[:], sraw[:], axis=mybir.AxisListType.XY)
            ps_eca = ps_pool.tile([C, 512], f32, tag="ps")
            nc.tensor.matmul(ps_eca[:, 0:1], trid_sb[:], pool_sum[:], start=True, stop=True)
            ca = small_pool.tile([C, 1], f32, tag="ca")
            nc.scalar.activation(ca[:], ps_eca[:, 0:1], Act.Sigmoid)
            w2p = w2p_pool.tile([C, OUT], f32, tag="w2p")
            nc.vector.tensor_scalar(r(w2p[:]), w2_sb[:], ca[:], None, Alu.mult)

            # ---- forward QKV GEMM: q and v in channels-on-partitions layout ----
            vpad = vpad_pool.tile([C, H + 4, W + 4], fp8, tag="vpad")
            nc.vector.memset(vpad[:, 0:2, :], 0.0)
            nc.vector.memset(vpad[:, H + 2 : H + 4, :], 0.0)
            nc.vector.memset(vpad[:, :, 0:2], 0.0)
            nc.vector.memset(vpad[:, :, W + 2 : W + 4], 0.0)
            q_t = []
            for c8 in range(NC8):
                psq = ps_pool.tile([C, 512], f32, tag="ps")
                nc.tensor.matmul(psq[:], r(wq_sb[:]), r(s_t[c8][:]), start=True, stop=True)
                qt = q_pool.tile([C, 512], f32, tag="q")
                if c8 % 2 == 0:
                    nc.scalar.copy(r(qt[:]), psq[:])
                else:
                    nc.vector.tensor_copy(r(qt[:]), psq[:])
                q_t.append(qt)
                psv = ps_pool.tile([C, 512], f32, tag="ps")
                nc.tensor.matmul(psv[:], r(wv_sb[:]), r(s_t[c8][:]), start=True, stop=True)
                nc.vector.tensor_copy(
                    vpad[:, 2 + 8 * c8 : 2 + 8 * c8 + 8, 2 : W + 2],
                    psv[:].rearrange("p (a b) -> p a b", a=8),
                )

            # ---- transposed GEMM: [kT | vT] chunks; exp(kT) -> eT, vT + ones col ----
            eT = eT_pool.tile([C, NJ, C], bf16, tag="eT")
            vT = vT_pool.tile([C, NJ, C + 1], bf16, tag="vT")
            nc.vector.memset(vT[:, :, C : C + 1], 1.0)
            for j in range(NJ):
                psT = psT_pool.tile([C, 2 * C], f32, tag="psT")
                lhs = s_t[j // 4][:, (j % 4) * 128 : (j % 4 + 1) * 128]
                nc.tensor.matmul(psT[:], r(lhs), r(wkv_sb[:]), start=True, stop=True)
                nc.scalar.activation(eT[:, j, :], psT[:, 0:C], Act.Exp)
                nc.vector.tensor_copy(vT[:, j, 0:C], psT[:, C : 2 * C])

            # ---- content lambda: CL[i, o] (+ row sums in col 128) ----
            ps_cl = psCL_pool.tile([C, C + 1], f32, tag="pscl")
            for j in range(NJ):
                nc.tensor.matmul(
                    ps_cl[:], eT[:, j, :], vT[:, j, :],
                    start=(j == 0), stop=(j == NJ - 1),
                )
            recip = small_pool.tile([C, 1], f32, tag="recip")
            nc.vector.reciprocal(recip[:], ps_cl[:, C : C + 1])
            cln_t = small_pool.tile([C, C], f32, tag="cln_t")
            nc.vector.tensor_scalar(cln_t[:], ps_cl[:, 0:C], recip[:], None, Alu.mult)
            cln = small_pool.tile([C, C], f32, tag="cln")
            nc.vector.tensor_tensor(r(cln[:]), cln_t[:], mask_sb[:], Alu.mult)

            # ---- per chunk: 5x5 conv + content output + r1 + out projection,
            # software-pipelined so out-proj(c8-1) overlaps the DVE r1 chain ----
            def emit_outproj(rt, c8):
                y0 = 8 * c8
                for m in range(OUT // C):
                    pso = ps_pool.tile([C, 512], f32, tag="ps")
                    nc.tensor.matmul(
                        pso[:], r(wf32b[:, m * C : (m + 1) * C]), r(rt[:]),
                        start=True, stop=False,
                    )
                    nc.tensor.matmul(
                        pso[:], r(w2p[:, m * C : (m + 1) * C]),
                        r(sraw[:, y0 : y0 + 8, :]),
                        start=False, stop=True,
                    )
                    stg = stage_pool.tile([C, 512], f32, tag="stage")
                    if m % 2 == 0:
                        nc.scalar.copy(stg[:], pso[:])
                    else:
                        nc.vector.tensor_copy(stg[:], pso[:])
                    eng = nc.sync if m % 2 == 0 else nc.scalar
                    eng.dma_start(
                        out_v[smp, m * C : (m + 1) * C,
                              c8 * 512 : (c8 + 1) * 512],
                        stg[:],
                    )

            prev = None
            for c8 in range(NC8):
                ps5 = ps_pool.tile([C, 512], f32, tag="ps")
                y0 = 8 * c8
                for p, ((dy0, dx0), (dy1, dx1)) in enumerate(_P5):
                    sl = vpad[:, y0 + dy0 : y0 + dy0 + 8, dx0 : dx0 + W]
                    delta = (dy1 - dy0) * (W + 4) + (dx1 - dx0)
                    nc.tensor.matmul(
                        ps5[:], d5_sb[:, 2 * p : 2 * p + 2, :],
                        pair_ap(sl, delta), start=(p == 0), stop=False,
                        perf_mode=DR,
                    )
                dy, dx = _P5_SINGLE
                sl = vpad[:, y0 + dy : y0 + dy + 8, dx : dx + W]
                nc.tensor.matmul(
                    ps5[:], d5_sb[:, 24:26, :], pair_ap(sl, -1),
                    start=False, stop=True, perf_mode=DR,
                )
                psc = ps_pool.tile([C, 512], f32, tag="ps")
                nc.tensor.matmul(psc[:], r(cln[:]), r(q_t[c8][:]), start=True, stop=True)
                tmp = tmp_pool.tile([C, 512], f32, tag="tmp")
                nc.vector.tensor_tensor(tmp[:], q_t[c8][:], ps5[:], Alu.mult)
                rt = r1_pool.tile([C, 512], f32, tag="r1")
                nc.vector.tensor_tensor(r(rt[:]), tmp[:], psc[:], Alu.add)
                if prev is not None:
                    emit_outproj(*prev)
                prev = (rt, c8)
            emit_outproj(*prev)

    nc.compile()
    return nc


def _get_nc():
    if "nc" not in _CACHE:
        _CACHE["nc"] = _build_nc()
    return _CACHE["nc"]


def _host_weights(cpe_w, qkv_w, rel_pos, conv1d_w, out_w):
    cpe_w = np.asarray(cpe_w, np.float32)
    qkv_w = np.asarray(qkv_w, np.float32)
    rel_pos = np.asarray(rel_pos, np.float32)
    conv1d_w = np.asarray(conv1d_w, np.float32)
    out_w = np.asarray(out_w, np.float32)

    import ml_dtypes
    idx = np.arange(C)

    # wfp8: d3 pair slots 0..7 | d5 pair slots 8..33
    wfp8 = np.zeros([C, 34, C], ml_dtypes.float8_e4m3)
    taps3 = [t for pr in _P3 for t in pr]
    for slot, (dy, dx) in enumerate(taps3):
        wfp8[idx, slot, idx] = cpe_w[:, 0, dy, dx]
    taps5 = [t for pr in _P5 for t in pr] + [_P5_SINGLE]
    for slot, (dy, dx) in enumerate(taps5):
        wfp8[idx, 8 + slot, idx] = rel_pos[idx % HD, dy, dx]

    dctr = np.zeros([C, C], np.float32)
    dctr[idx, idx] = 1.0 + cpe_w[:, 0, 1, 1]  # residual + center tap, exact

    wq = qkv_w[0:C, :].T
    wv = qkv_w[2 * C : 3 * C, :].T
    wkv = qkv_w[C : 3 * C, :].T
    w1 = out_w[:, 0:C].T
    w2 = out_w[:, C : 2 * C].T

    mask = np.zeros([C, C], np.float32)
    for h in range(NH):
        mask[h * HD : (h + 1) * HD, h * HD : (h + 1) * HD] = SCALING

    trid = np.zeros([C, C], np.float32)
    trid[idx[:-1], idx[:-1] + 1] = conv1d_w[0]  # pool[c-1] contributes to ca[c]
    trid[idx, idx] = conv1d_w[1]
    trid[idx[1:], idx[1:] - 1] = conv1d_w[2]
    trid *= 1.0 / N

    wf32a = np.ascontiguousarray(
        np.concatenate([dctr, wq, wv, wkv], axis=1), np.float32)
    wf32b = np.ascontiguousarray(
        np.concatenate([w1, w2, mask, trid], axis=1), np.float32)
    return dict(wfp8=wfp8, wf32a=wf32a, wf32b=wf32b)


def kernel(src, cpe_w, qkv_w, rel_pos, conv1d_w, out_w):
    from concourse.bass_utils import run_bass_kernel_spmd

    import ml_dtypes

    src = np.asarray(src, np.float32)
    w = _host_weights(cpe_w, qkv_w, rel_pos, conv1d_w, out_w)
    nc = _get_nc()
    src8 = np.zeros([B, C, H + 2, W + 2], ml_dtypes.float8_e4m3)
    src8[:, :, 1 : H + 1, 1 : W + 1] = src
    in_maps = [
        {"src": np.ascontiguousarray(src[i * SPC : (i + 1) * SPC]),
         "src8": np.ascontiguousarray(src8[i * SPC : (i + 1) * SPC]), **w}
        for i in range(NCORES)
    ]
    trace = bool(os.environ.get("BASS_TRACE"))
    res = run_bass_kernel_spmd(nc, in_maps, list(range(NCORES)), trace=trace)
    _CACHE["last_result"] = res
    out = np.concatenate([res.results[i]["out"] for i in range(NCORES)], axis=0)
    return out
